# revision 2
# baseline (speedup 1.0000x reference)
"""Trainium2 Bass kernel for nn_AttentionModule_66537633349985 (segment attention pooling).

Same math as the baseline kernel.py (softmax-cancelled b_a and segment-max):
per 4096-node macro-tile, H-on-partitions bf16 layout,
    t    = W_c @ embT            (PE, bf16, 1024-wide matmuls)
    tT   = tanh(t + b_c)         (ACT, psum -> sbuf bf16, 1024-wide)
    s    = w_a . tT              (PE; score row 32k packs chunk pair 2k,2k+1
                                  as one 1024-wide matmul via tile_position)
    e    = exp(s)                (ACT, one [128,1024] pass, junk rows unused)
    eb   = broadcast e           (row 0 via gpsimd partition_broadcast - the
                                  only partition it supports; rows 1-3 via PE
                                  rank-1 ones x e_row matmuls)
    P[:, c] = sum_n emb*eb       (DVE scalar_tensor_tensor, fused multiply +
                                  512-block sum via accum_out)
Host epilogue: whole 512-blocks from P, boundary blocks recomputed from the
dumped e, denominators via bincount. b_a and the segment max cancel in the
softmax ratio (scores bounded by sum|w_a| ~ 6, exp cannot overflow f32).
"""
import numpy as np

import concourse.bass as bass
import concourse.bacc as bacc
import concourse.tile as tile
import concourse.mybir as mybir
from concourse.bass_utils import run_bass_kernel_spmd

H = 128            # hidden dim
B = 1024           # number of graphs
NCORES = 8
TM = 4096          # nodes per macro-tile
NCH = TM // 512    # 512-node chunks per macro-tile
NPAIR = NCH // 2   # chunk pairs (score rows)
BLK = 512          # block size of the on-chip partial sums

f32 = mybir.dt.float32
f32r = mybir.dt.float32r
bf16 = mybir.dt.bfloat16

_BUILD_CACHE: dict = {}


def build_bass(L: int, repeat: int = 1, hw_loop: int = 0,
               use_sel: bool = True, gp_row0: bool = True) -> "bacc.Bacc":
    """Per-core Bass program for an [H, L] H-major bf16 embedding shard."""
    wide_mm = False
    key = (L, repeat, hw_loop, use_sel, gp_row0)
    if key in _BUILD_CACHE:
        return _BUILD_CACHE[key]
    assert L % TM == 0
    nmacro = L // TM

    nc = bacc.Bacc("TRN2", target_bir_lowering=False, debug=False)

    embT_d = nc.dram_tensor("embT", [H, L], bf16, kind="ExternalInput")
    W_d = nc.dram_tensor("W", [H, H], bf16, kind="ExternalInput")   # W_c.T
    wa_d = nc.dram_tensor("wa", [H, 1], f32, kind="ExternalInput")
    bc_d = nc.dram_tensor("bc", [H, 1], f32, kind="ExternalInput")
    sel_d = nc.dram_tensor("sel", [H, NPAIR * H], bf16, kind="ExternalInput")
    P_d = nc.dram_tensor("P", [nmacro, H, NCH], f32, kind="ExternalOutput")
    e_d = nc.dram_tensor("e", [nmacro, NCH, 512], bf16, kind="ExternalOutput")

    Tanh = mybir.ActivationFunctionType.Tanh
    Exp = mybir.ActivationFunctionType.Exp
    Mult = mybir.AluOpType.mult

    import contextlib

    with tile.TileContext(nc) as tc:
        with (
            tc.tile_pool(name="const", bufs=1) as cpool,
            tc.tile_pool(name="sbuf", bufs=4) as pool,
            tc.tile_pool(name="small", bufs=8) as spool,
            tc.tile_pool(name="pt", bufs=2, space="PSUM") as pt_pool,
            tc.tile_pool(name="psc", bufs=1, space="PSUM") as psc_pool,
            tc.tile_pool(name="pe", bufs=1, space="PSUM") as pe_pool,
        ):
            W_sb = cpool.tile([H, H], bf16)
            wa_sb = cpool.tile([H, 1], f32)
            wa_bf = cpool.tile([H, 1], bf16)
            bc_sb = cpool.tile([H, 1], f32)
            ones_f = cpool.tile([H, H], f32)
            ones_bf = cpool.tile([H, H], bf16)
            nc.sync.dma_start(W_sb[:], W_d[:])
            nc.sync.dma_start(wa_sb[:], wa_d[:])
            nc.sync.dma_start(bc_sb[:], bc_d[:])
            nc.vector.tensor_copy(wa_bf[:], wa_sb[:])
            nc.vector.memset(ones_f[:], 1.0)
            nc.vector.tensor_copy(ones_bf[:], ones_f[:])
            sel_sb = cpool.tile([H, NPAIR * H], bf16)
            nc.sync.dma_start(sel_sb[:], sel_d[:])
            # single psum_s tile reused across macros; memset once so the
            # junk rows are exp(0)=1 (finite) from the first macro on
            psum_s = psc_pool.tile([H, 1024], f32)
            nc.vector.memset(psum_s[:], 0.0)

            loop_cm = (tc.For_i(0, hw_loop, 1) if hw_loop
                       else contextlib.nullcontext())
            with loop_cm:
                for m in [mm for _ in range(repeat) for mm in range(nmacro)]:
                    emb_sb = pool.tile([H, TM], bf16, tag="emb")
                    nc.sync.dma_start(emb_sb[:],
                                      embT_d[:, m * TM:(m + 1) * TM])

                    # t = W_c @ embT; tanh per 1024 columns
                    tT_sb = pool.tile([H, TM], bf16, tag="tT")
                    for h in range(TM // 1024):
                        psum_t = pt_pool.tile([H, 1024], f32, tag="pt")
                        if wide_mm:
                            nc.tensor.matmul(
                                psum_t[:],
                                W_sb[:],
                                emb_sb[:, h * 1024:(h + 1) * 1024],
                                start=True, stop=True,
                            )
                        else:
                            for j in range(2):
                                nc.tensor.matmul(
                                    psum_t[:, j * 512:(j + 1) * 512],
                                    W_sb[:],
                                    emb_sb[:, (2 * h + j) * 512:
                                           (2 * h + j + 1) * 512],
                                    start=True, stop=True,
                                )
                        nc.scalar.activation(
                            tT_sb[:, h * 1024:(h + 1) * 1024], psum_t[:],
                            Tanh, bias=bc_sb[:])

                    # scores: row 32k of psum_s = chunks (2k, 2k+1)
                    for k in range(NPAIR):
                        for d in range(2):
                            nc.tensor.matmul(
                                psum_s[32 * k:32 * k + 1,
                                       d * 512:(d + 1) * 512],
                                wa_bf[:],
                                tT_sb[:, (2 * k + d) * 512:
                                       (2 * k + d + 1) * 512],
                                start=True, stop=True,
                                tile_position=(0, 32 * k),
                            )
                    e_sb = spool.tile([H, 1024], bf16, tag="e")
                    nc.scalar.activation(e_sb[:], psum_s[:], Exp)
                    nc.sync.dma_start(
                        e_d[m].rearrange("(k d) x -> k d x", k=NPAIR, d=2),
                        e_sb[0:H:32, :].rearrange("p (d x) -> p d x", d=2))
                    # broadcast + fused pooling per chunk pair
                    P_sb = spool.tile([H, NCH], f32, tag="P")
                    for k in range(NPAIR):
                        if k == 0 and gp_row0:
                            # pair 0 lives on partition 0: gpsimd broadcast
                            eb_sb = spool.tile([H, 1024], bf16, tag="eb")
                            nc.gpsimd.partition_broadcast(
                                eb_sb[:], e_sb[0:1, :])
                            in1 = eb_sb
                        else:
                            # row 32k of sel_k is ones, rest zeros:
                            # out[m, c] = e_sb[32k, c] for every m
                            psum_eb = pe_pool.tile([H, 1024], f32, tag="pe")
                            for d in range(2):
                                nc.tensor.matmul(
                                    psum_eb[:, d * 512:(d + 1) * 512],
                                    sel_sb[:, k * H:(k + 1) * H],
                                    e_sb[:, d * 512:(d + 1) * 512],
                                    start=True, stop=True,
                                )
                            in1 = psum_eb
                        for d in range(2):
                            c = 2 * k + d
                            scr = spool.tile([H, 512], bf16, tag="scr")
                            nc.vector.scalar_tensor_tensor(
                                out=scr[:],
                                in0=emb_sb[:, c * 512:(c + 1) * 512],
                                scalar=1.0,
                                in1=in1[:, d * 512:(d + 1) * 512],
                                op0=Mult,
                                op1=Mult,
                                accum_out=P_sb[:, c:c + 1],
                            )
                    nc.sync.dma_start(P_d[m], P_sb[:])

    nc.compile()
    _BUILD_CACHE[key] = nc
    return nc


def make_sel():
    import ml_dtypes
    sel = np.zeros((H, NPAIR * H), dtype=ml_dtypes.bfloat16)
    for k in range(NPAIR):
        sel[32 * k, k * H:(k + 1) * H] = 1.0
    return sel


def kernel(**inputs) -> np.ndarray:
    import ml_dtypes
    emb = np.ascontiguousarray(np.asarray(inputs["embeddings"], dtype=np.float32))
    batch = np.asarray(inputs["batch"]).astype(np.int64)
    W_c = np.asarray(inputs["W_c"], dtype=np.float32)
    b_c = np.asarray(inputs["b_c"], dtype=np.float32)
    w_a = np.asarray(inputs["w_a"], dtype=np.float32)
    # b_a cancels in the softmax; unused.

    N = emb.shape[0]
    assert N % NCORES == 0
    SH = N // NCORES                      # nodes per core
    L = (SH // TM) * TM                   # whole macro-tiles only; the short
    TAIL = SH - L                         # per-core tail is done on the host

    embT = np.empty((NCORES, H, L), dtype=ml_dtypes.bfloat16)
    for c in range(NCORES):
        embT[c][:] = emb[c * SH:c * SH + L].T.astype(ml_dtypes.bfloat16)

    nc = build_bass(L)
    Wt = np.ascontiguousarray(W_c.T.astype(ml_dtypes.bfloat16))
    wa_col = np.ascontiguousarray(w_a[:, None])
    bc_col = np.ascontiguousarray(b_c[:, None])
    sel = make_sel()
    in_maps = [
        {"embT": embT[c], "W": Wt, "wa": wa_col, "bc": bc_col, "sel": sel}
        for c in range(NCORES)
    ]
    res = run_bass_kernel_spmd(nc, in_maps, core_ids=list(range(NCORES)))

    num = np.zeros((B, H), dtype=np.float64)
    e_global = np.empty(N, dtype=np.float32)
    nblk_real = L // BLK
    if TAIL:
        # per-core tail nodes: full forward on the host (tiny)
        for c in range(NCORES):
            g0 = c * SH + L
            et = emb[g0:g0 + TAIL]
            st = np.tanh(et @ W_c.T + b_c) @ w_a
            e_global[g0:g0 + TAIL] = np.exp(st)
            segs = batch[g0:g0 + TAIL]
            for s in np.unique(segs):
                msk = segs == s
                num[s] += e_global[g0:g0 + TAIL][msk] @ et[msk]
    for c in range(NCORES):
        P = res.results[c]["P"]                          # [nmacro, H, NCH]
        e_flat = np.asarray(res.results[c]["e"],
                            dtype=np.float32).reshape(-1)    # [L]
        e_global[c * SH:c * SH + L] = e_flat
        P_flat = np.moveaxis(P, 1, 0).reshape(H, -1)     # [H, L//BLK]
        for b in range(nblk_real):
            g0 = c * SH + BLK * b
            g1 = g0 + BLK
            s0 = batch[g0]
            s1 = batch[g1 - 1]
            if s0 == s1:
                num[s0] += P_flat[:, b]
            else:
                # boundary block: recompute exactly on host per segment run
                segs = batch[g0:g1]
                eb = e_flat[BLK * b: BLK * b + (g1 - g0)].astype(np.float64)
                cuts = np.concatenate(
                    [[0], np.flatnonzero(np.diff(segs)) + 1, [g1 - g0]])
                for r in range(len(cuts) - 1):
                    r0, r1 = cuts[r], cuts[r + 1]
                    num[segs[r0]] += eb[r0:r1] @ emb[g0 + r0: g0 + r1]
    den = np.bincount(batch, weights=e_global, minlength=B)
    den[den == 0.0] = 1.0          # empty segments -> 0 output (matches reference)
    return (num / den[:, None]).astype(np.float32)


# revision 3
# speedup vs baseline: 1.3245x; 1.3245x over previous
"""Trainium2 Bass kernel for nn_AttentionModule_66537633349985 (segment attention pooling).

Same math as the baseline kernel.py (softmax-cancelled b_a and segment-max):
per 4096-node macro-tile, H-on-partitions bf16 layout,
    t    = W_c @ embT            (PE, bf16, 1024-wide matmuls)
    tT   = tanh(t + b_c)         (ACT, psum -> sbuf bf16, 1024-wide)
    s    = w_a . tT              (PE; score row 32k packs chunk pair 2k,2k+1
                                  as one 1024-wide matmul via tile_position)
    e    = exp(s)                (ACT, one [128,1024] pass, junk rows unused)
    eb   = broadcast e           (row 0 via gpsimd partition_broadcast - the
                                  only partition it supports; rows 1-3 via PE
                                  rank-1 ones x e_row matmuls)
    P[:, c] = sum_n emb*eb       (DVE scalar_tensor_tensor, fused multiply +
                                  512-block sum via accum_out)
Host epilogue: whole 512-blocks from P, boundary blocks recomputed from the
dumped e, denominators via bincount. b_a and the segment max cancel in the
softmax ratio (scores bounded by sum|w_a| ~ 6, exp cannot overflow f32).
"""
import numpy as np

import concourse.bass as bass
import concourse.bacc as bacc
import concourse.tile as tile
import concourse.mybir as mybir
from concourse.bass_utils import run_bass_kernel_spmd

H = 128            # hidden dim
B = 1024           # number of graphs
NCORES = 8
TM = 4096          # nodes per macro-tile
NCH = TM // 512    # 512-node chunks per macro-tile
NPAIR = NCH // 2   # chunk pairs (score rows)
BLK = 512          # block size of the on-chip partial sums

f32 = mybir.dt.float32
f32r = mybir.dt.float32r
bf16 = mybir.dt.bfloat16

_BUILD_CACHE: dict = {}


def build_bass(L: int, repeat: int = 1, hw_loop: int = 0,
               use_sel: bool = True, gp_row0: bool = True) -> "bacc.Bacc":
    """Per-core Bass program for an [H, L] H-major bf16 embedding shard."""
    wide_mm = False
    key = (L, repeat, hw_loop, use_sel, gp_row0)
    if key in _BUILD_CACHE:
        return _BUILD_CACHE[key]
    assert L % TM == 0
    nmacro = L // TM

    nc = bacc.Bacc("TRN2", target_bir_lowering=False, debug=False)

    embT_d = nc.dram_tensor("embT", [H, L], bf16, kind="ExternalInput")
    W_d = nc.dram_tensor("W", [H, H], bf16, kind="ExternalInput")   # W_c.T
    wa_d = nc.dram_tensor("wa", [H, 1], f32, kind="ExternalInput")
    bc_d = nc.dram_tensor("bc", [H, 1], f32, kind="ExternalInput")
    sel_d = nc.dram_tensor("sel", [H, NPAIR * H], bf16, kind="ExternalInput")
    P_d = nc.dram_tensor("P", [nmacro, H, NCH], f32, kind="ExternalOutput")
    e_d = nc.dram_tensor("e", [nmacro, NCH, 512], bf16, kind="ExternalOutput")

    Tanh = mybir.ActivationFunctionType.Tanh
    Exp = mybir.ActivationFunctionType.Exp
    Mult = mybir.AluOpType.mult

    import contextlib

    with tile.TileContext(nc) as tc:
        with (
            tc.tile_pool(name="const", bufs=1) as cpool,
            tc.tile_pool(name="sbuf", bufs=4) as pool,
            tc.tile_pool(name="small", bufs=8) as spool,
            tc.tile_pool(name="pt", bufs=2, space="PSUM") as pt_pool,
            tc.tile_pool(name="psc", bufs=1, space="PSUM") as psc_pool,
            tc.tile_pool(name="pe", bufs=2, space="PSUM") as pe_pool,
        ):
            W_sb = cpool.tile([H, H], bf16)
            wa_sb = cpool.tile([H, 1], f32)
            wa_bf = cpool.tile([H, 1], bf16)
            bc_sb = cpool.tile([H, 1], f32)
            ones_f = cpool.tile([H, H], f32)
            ones_bf = cpool.tile([H, H], bf16)
            nc.sync.dma_start(W_sb[:], W_d[:])
            nc.sync.dma_start(wa_sb[:], wa_d[:])
            nc.sync.dma_start(bc_sb[:], bc_d[:])
            nc.vector.tensor_copy(wa_bf[:], wa_sb[:])
            nc.vector.memset(ones_f[:], 1.0)
            nc.vector.tensor_copy(ones_bf[:], ones_f[:])
            sel_sb = cpool.tile([H, NPAIR * H], bf16)
            nc.sync.dma_start(sel_sb[:], sel_d[:])
            # single psum_s tile reused across macros; memset once so the
            # junk rows are exp(0)=1 (finite) from the first macro on
            psum_s = psc_pool.tile([H, 1024], f32)
            nc.vector.memset(psum_s[:], 0.0)

            loop_cm = (tc.For_i(0, hw_loop, 1) if hw_loop
                       else contextlib.nullcontext())
            with loop_cm:
                for m in [mm for _ in range(repeat) for mm in range(nmacro)]:
                    emb_sb = pool.tile([H, TM], bf16, tag="emb")
                    nc.sync.dma_start(emb_sb[:],
                                      embT_d[:, m * TM:(m + 1) * TM])

                    # t = W_c @ embT; tanh per 1024 columns
                    tT_sb = pool.tile([H, TM], bf16, tag="tT")
                    for h in range(TM // 1024):
                        psum_t = pt_pool.tile([H, 1024], f32, tag="pt")
                        if wide_mm:
                            nc.tensor.matmul(
                                psum_t[:],
                                W_sb[:],
                                emb_sb[:, h * 1024:(h + 1) * 1024],
                                start=True, stop=True,
                            )
                        else:
                            for j in range(2):
                                nc.tensor.matmul(
                                    psum_t[:, j * 512:(j + 1) * 512],
                                    W_sb[:],
                                    emb_sb[:, (2 * h + j) * 512:
                                           (2 * h + j + 1) * 512],
                                    start=True, stop=True,
                                )
                        nc.scalar.activation(
                            tT_sb[:, h * 1024:(h + 1) * 1024], psum_t[:],
                            Tanh, bias=bc_sb[:])

                    # scores: row 32k of psum_s = chunks (2k, 2k+1)
                    for k in range(NPAIR):
                        for d in range(2):
                            nc.tensor.matmul(
                                psum_s[32 * k:32 * k + 1,
                                       d * 512:(d + 1) * 512],
                                wa_bf[:],
                                tT_sb[:, (2 * k + d) * 512:
                                       (2 * k + d + 1) * 512],
                                start=True, stop=True,
                                tile_position=(0, 32 * k),
                            )
                    e_sb = spool.tile([H, 1024], bf16, tag="e")
                    nc.scalar.activation(e_sb[:], psum_s[:], Exp)
                    nc.sync.dma_start(
                        e_d[m].rearrange("(k d) x -> k d x", k=NPAIR, d=2),
                        e_sb[0:H:32, :].rearrange("p (d x) -> p d x", d=2))
                    # broadcast + fused pooling per chunk pair
                    P_sb = spool.tile([H, NCH], f32, tag="P")
                    for k in range(NPAIR):
                        if k == 0 and gp_row0:
                            # pair 0 lives on partition 0: gpsimd broadcast
                            eb_sb = spool.tile([H, 1024], bf16, tag="eb")
                            nc.gpsimd.partition_broadcast(
                                eb_sb[:], e_sb[0:1, :])
                            in1 = eb_sb
                        else:
                            in1 = None
                        for d in range(2):
                            c = 2 * k + d
                            if in1 is None:
                                # row 32k of sel_k is ones, rest zeros:
                                # out[m, c] = e_sb[32k, c] for every m;
                                # [H,512] tiles rotate so the next broadcast
                                # overlaps this chunk's pooling pass
                                psum_eb = pe_pool.tile([H, 512], f32,
                                                       tag="pe")
                                nc.tensor.matmul(
                                    psum_eb[:],
                                    sel_sb[:, k * H:(k + 1) * H],
                                    e_sb[:, d * 512:(d + 1) * 512],
                                    start=True, stop=True,
                                )
                                src1 = psum_eb[:]
                            else:
                                src1 = in1[:, d * 512:(d + 1) * 512]
                            scr = spool.tile([H, 512], bf16, tag="scr")
                            nc.vector.scalar_tensor_tensor(
                                out=scr[:],
                                in0=emb_sb[:, c * 512:(c + 1) * 512],
                                scalar=1.0,
                                in1=src1,
                                op0=Mult,
                                op1=Mult,
                                accum_out=P_sb[:, c:c + 1],
                            )
                    nc.sync.dma_start(P_d[m], P_sb[:])

    nc.compile()
    _BUILD_CACHE[key] = nc
    return nc


def make_sel():
    import ml_dtypes
    sel = np.zeros((H, NPAIR * H), dtype=ml_dtypes.bfloat16)
    for k in range(NPAIR):
        sel[32 * k, k * H:(k + 1) * H] = 1.0
    return sel


def kernel(**inputs) -> np.ndarray:
    import ml_dtypes
    emb = np.ascontiguousarray(np.asarray(inputs["embeddings"], dtype=np.float32))
    batch = np.asarray(inputs["batch"]).astype(np.int64)
    W_c = np.asarray(inputs["W_c"], dtype=np.float32)
    b_c = np.asarray(inputs["b_c"], dtype=np.float32)
    w_a = np.asarray(inputs["w_a"], dtype=np.float32)
    # b_a cancels in the softmax; unused.

    N = emb.shape[0]
    assert N % NCORES == 0
    SH = N // NCORES                      # nodes per core
    L = (SH // TM) * TM                   # whole macro-tiles only; the short
    TAIL = SH - L                         # per-core tail is done on the host

    embT = np.empty((NCORES, H, L), dtype=ml_dtypes.bfloat16)
    for c in range(NCORES):
        embT[c][:] = emb[c * SH:c * SH + L].T.astype(ml_dtypes.bfloat16)

    nc = build_bass(L)
    Wt = np.ascontiguousarray(W_c.T.astype(ml_dtypes.bfloat16))
    wa_col = np.ascontiguousarray(w_a[:, None])
    bc_col = np.ascontiguousarray(b_c[:, None])
    sel = make_sel()
    in_maps = [
        {"embT": embT[c], "W": Wt, "wa": wa_col, "bc": bc_col, "sel": sel}
        for c in range(NCORES)
    ]
    res = run_bass_kernel_spmd(nc, in_maps, core_ids=list(range(NCORES)))

    num = np.zeros((B, H), dtype=np.float64)
    e_global = np.empty(N, dtype=np.float32)
    nblk_real = L // BLK
    if TAIL:
        # per-core tail nodes: full forward on the host (tiny)
        for c in range(NCORES):
            g0 = c * SH + L
            et = emb[g0:g0 + TAIL]
            st = np.tanh(et @ W_c.T + b_c) @ w_a
            e_global[g0:g0 + TAIL] = np.exp(st)
            segs = batch[g0:g0 + TAIL]
            for s in np.unique(segs):
                msk = segs == s
                num[s] += e_global[g0:g0 + TAIL][msk] @ et[msk]
    for c in range(NCORES):
        P = res.results[c]["P"]                          # [nmacro, H, NCH]
        e_flat = np.asarray(res.results[c]["e"],
                            dtype=np.float32).reshape(-1)    # [L]
        e_global[c * SH:c * SH + L] = e_flat
        P_flat = np.moveaxis(P, 1, 0).reshape(H, -1)     # [H, L//BLK]
        for b in range(nblk_real):
            g0 = c * SH + BLK * b
            g1 = g0 + BLK
            s0 = batch[g0]
            s1 = batch[g1 - 1]
            if s0 == s1:
                num[s0] += P_flat[:, b]
            else:
                # boundary block: recompute exactly on host per segment run
                segs = batch[g0:g1]
                eb = e_flat[BLK * b: BLK * b + (g1 - g0)].astype(np.float64)
                cuts = np.concatenate(
                    [[0], np.flatnonzero(np.diff(segs)) + 1, [g1 - g0]])
                for r in range(len(cuts) - 1):
                    r0, r1 = cuts[r], cuts[r + 1]
                    num[segs[r0]] += eb[r0:r1] @ emb[g0 + r0: g0 + r1]
    den = np.bincount(batch, weights=e_global, minlength=B)
    den[den == 0.0] = 1.0          # empty segments -> 0 output (matches reference)
    return (num / den[:, None]).astype(np.float32)
